# revision 2
# baseline (speedup 1.0000x reference)
"""Trainium2 Bass kernel for a 16-head causal self-attention block.

Reference computation (B=1, S=4096, H=2048, 16 heads x 128 dim, fp32):
    qkv = x @ w_qkv.T            # [S, 6144]
    q, k = rope(q), rope(k)      # half-split rope
    attn = causal_softmax(q k^T / sqrt(128)) @ v
    out  = attn @ w_o.T          # [S, 2048]

Sharding: tensor-parallel over heads.  Each of the 8 cores owns 2 heads:
it computes its slice of the QKV projection (768 rows), attention for its
2 heads, and a partial o_proj ([S, 2048], bf16); the host sums the 8
partials in f32.

Dataflow per core (matmul operands bf16/fp16, accumulation/softmax fp32):
  phase 1 (QKV+rope), per s-tile of 512: w/x chunk DMAs interleaved so the
    first sweep starts as soon as pair 0 lands; Q, K, V matmul sweeps with
    rope fused into the Q/K epilogues (rotate_half = signed-permutation
    matmul on the PE); V stored fp16.
  phase 2 (attention + o_proj software-pipelined), per q-tile of 512:
    causal k-chunks of 128 with variable-width matmuls on the diagonal
    (blocks strictly above the diagonal are never computed), exp -> fp16,
    a single shared [128,128] triangular mask for diagonal blocks, softmax
    denominators accumulated in fp16 on DVE (2x mode) into two alternating
    accumulators, folded across partitions by a ones-matmul per tile;
    the previous tile's fold/normalize and o_proj groups are interleaved
    into the next tile's chunk stream so the PE never idles; o_proj
    staging copies are split DVE/ACT to balance engine load.
"""

import numpy as np

import concourse.bass as bass
import concourse.mybir as mybir
import concourse.tile as tile
from concourse import bacc
from concourse.bass_utils import run_bass_kernel_spmd

F32 = mybir.dt.float32
BF16 = mybir.dt.bfloat16
FP16 = mybir.dt.float16

S = 4096
H = 2048
DH = 128
NH = 16
NCORES = 8
HPC = NH // NCORES          # 2 heads per core
OLOC = HPC * DH             # 256 local o-channels per q/k/v group
P = 128
ST1 = 512                   # phase-1 s-tile width
NHT = H // P                # 16 h-chunks
QT = 512                    # phase-2 q-tile width
NQT = S // QT               # 8 q-tiles
NKC = S // P                # 32 k-chunks
NOG = (QT // P) * (H // QT)  # 16 o_proj groups per q-tile
SCALE = 1.0 / float(np.sqrt(np.float32(DH)))

_PROGRAM = None


def _build_body(tc):
    nc = tc.nc

    xT = nc.dram_tensor("xT", [H, S], BF16, kind="ExternalInput").ap()
    wqkvT = nc.dram_tensor("wqkvT", [H, 3 * OLOC], BF16, kind="ExternalInput").ap()
    woT = nc.dram_tensor("woT", [OLOC, H], BF16, kind="ExternalInput").ap()
    rope = nc.dram_tensor("rope", [P, 2, S], F32, kind="ExternalInput").ap()
    swapj = nc.dram_tensor("swapj", [P, P], BF16, kind="ExternalInput").ap()
    ones16 = nc.dram_tensor("ones16", [P, P], FP16, kind="ExternalInput").ap()
    maskT = nc.dram_tensor("maskT", [P, HPC, P], FP16, kind="ExternalInput").ap()
    maskF = nc.dram_tensor("maskF", [P, 2, HPC, QT], FP16, kind="ExternalInput").ap()
    out = nc.dram_tensor("out", [S, H], BF16, kind="ExternalOutput").ap()

    xT_v = xT.rearrange("(t p) s -> p t s", p=P)        # [128, 16, 4096]
    wq_v = wqkvT.rearrange("(t p) o -> p t o", p=P)     # [128, 16, 768]
    woT_v = woT.rearrange("(t p) h -> p t h", p=P)      # [128, 2, 2048]

    with tc.tile_pool(name="resident", bufs=1) as resident:
        # d-major Q^T/K^T: [128 d, head, s]; s-major V: [128 s, k-chunk, 256]
        QT_sb = resident.tile([P, HPC, S], BF16)
        KT_sb = resident.tile([P, HPC, S], BF16)
        V_sb = resident.tile([P, NKC, OLOC], FP16)
        A_sb = resident.tile([P, HPC, S], BF16)          # normalized attn^T
        woT_sb = resident.tile([P, HPC, H], BF16)
        ones_sb = resident.tile([P, P], FP16)
        maskT_sb = resident.tile([P, HPC, P], FP16)
        maskF_sb = resident.tile([P, 2, HPC, QT], FP16)

        # ---------------- phase 1: QKV projection + rope ----------------
        with (
            tc.tile_pool(name="p1w", bufs=1) as p1w,
            tc.tile_pool(name="p1x", bufs=36) as p1x,
            tc.tile_pool(name="p1tab", bufs=3) as p1tab,
            tc.tile_pool(name="p1tmp", bufs=4) as p1tmp,
            tc.tile_pool(name="p1ps", bufs=1, space="PSUM") as p1ps,
            tc.tile_pool(name="p1rot", bufs=2, space="PSUM") as p1rot,
        ):
            J_sb = p1w.tile([P, P], BF16)
            nc.sync.dma_start(J_sb, swapj)
            wT_sb = p1w.tile([P, NHT, 3 * OLOC], BF16)

            def rope_block(blk, dst, cos, sin):
                t1 = p1tmp.tile([P, ST1], F32, tag="t1", name="t1")
                t2 = p1tmp.tile([P, ST1], BF16, tag="t2", name="t2")
                nc.vector.tensor_mul(t1, blk, cos)
                nc.vector.tensor_mul(t2, blk, sin)
                rot = p1rot.tile([P, ST1], F32, tag="rot", name="rot")
                nc.tensor.matmul(rot, lhsT=J_sb, rhs=t2, start=True, stop=True)
                nc.vector.tensor_add(dst, t1, rot)

            for st in range(S // ST1):
                s0 = st * ST1
                tab = p1tab.tile([P, 2, ST1], F32, tag="tab")
                nc.sync.dma_start(tab, rope[:, :, s0:s0 + ST1])
                xts = []
                for ht in range(NHT):
                    if st == 0:
                        nc.sync.dma_start(wT_sb[:, ht, :], wq_v[:, ht, :])
                    xt = p1x.tile([P, ST1], BF16, tag="xt", name=f"xt{st}_{ht}")
                    nc.sync.dma_start(xt, xT_v[:, ht, s0:s0 + ST1])
                    xts.append(xt)
                if st == 0:
                    # prefetch phase-2 constants while phase 1 computes
                    nc.sync.dma_start(ones_sb, ones16)
                    nc.sync.dma_start(maskT_sb, maskT)
                    nc.sync.dma_start(maskF_sb, maskF)
                    nc.sync.dma_start(woT_sb, woT_v)
                cos = tab[:, 0, :]
                sin = tab[:, 1, :]

                # --- Q sweep + rope (DMA-paced on s-tile 0) ---
                pq = p1ps.tile([P, 2 * ST1], F32, tag="pq", name=f"pq{st}")
                for ht in range(NHT):
                    for h in range(HPC):
                        nc.tensor.matmul(
                            pq[:, h * ST1:(h + 1) * ST1],
                            lhsT=wT_sb[:, ht, h * P:(h + 1) * P],
                            rhs=xts[ht],
                            start=ht == 0, stop=ht == NHT - 1,
                        )
                for h in range(HPC):
                    rope_block(pq[:, h * ST1:(h + 1) * ST1],
                               QT_sb[:, h, s0:s0 + ST1], cos, sin)

                # --- K sweep + rope ---
                pk = p1ps.tile([P, 2 * ST1], F32, tag="pk", name=f"pk{st}")
                for ht in range(NHT):
                    for h in range(HPC):
                        nc.tensor.matmul(
                            pk[:, h * ST1:(h + 1) * ST1],
                            lhsT=wT_sb[:, ht, OLOC + h * P:OLOC + (h + 1) * P],
                            rhs=xts[ht],
                            start=ht == 0, stop=ht == NHT - 1,
                        )
                for h in range(HPC):
                    rope_block(pk[:, h * ST1:(h + 1) * ST1],
                               KT_sb[:, h, s0:s0 + ST1], cos, sin)

                # --- V sweep ---
                # [128, 1024]: s-subs 0,1 share zero-region/bank 0; 2,3 share 1
                pv = p1ps.tile([P, 2 * ST1], F32, tag="pv", name=f"pv{st}")
                for ht in range(NHT):
                    for sub in range(ST1 // P):
                        nc.tensor.matmul(
                            pv[:, sub * OLOC:(sub + 1) * OLOC],
                            lhsT=xts[ht][:, sub * P:(sub + 1) * P],
                            rhs=wT_sb[:, ht, 2 * OLOC:3 * OLOC],
                            start=(ht == 0) and sub % 2 == 0,
                            stop=(ht == NHT - 1) and sub % 2 == 1,
                        )
                for sub in range(ST1 // P):
                    nc.scalar.activation(
                        V_sb[:, st * (ST1 // P) + sub, :],
                        pv[:, sub * OLOC:(sub + 1) * OLOC],
                        mybir.ActivationFunctionType.Copy,
                    )

        # ---------------- phase 2: attention + o_proj ----------------
        with (
            tc.tile_pool(name="p2e", bufs=3) as p2e,
            tc.tile_pool(name="p2acc", bufs=2) as p2acc,
            tc.tile_pool(name="p2rec", bufs=2) as p2rec,
            tc.tile_pool(name="p2st", bufs=4) as p2st,
            tc.tile_pool(name="p2sc", bufs=2, space="PSUM") as p2sc,
            tc.tile_pool(name="p2pv", bufs=1, space="PSUM") as p2pv,
            tc.tile_pool(name="p2po", bufs=2, space="PSUM") as p2po,
        ):
            state = {}
            ncopy = [0]

            def chunk_off(t, c):
                j = c - 4 * t
                if j < 0 or (t == 0 and c < 2):
                    return 0
                return P * j

            def chunk_front(t, c):
                # scores -> exp -> (mask) -> denominator accumulate
                q0 = t * QT
                off = chunk_off(t, c)
                sc = p2sc.tile([P, HPC, QT], F32, tag="sc", name=f"sc{t}_{c}")
                for h in range(HPC):
                    nc.tensor.matmul(
                        sc[:, h, off:],
                        lhsT=KT_sb[:, h, c * P:(c + 1) * P],
                        rhs=QT_sb[:, h, q0 + off:q0 + QT],
                        start=True, stop=True,
                    )
                e = p2e.tile([P, HPC, QT], FP16, tag="e", name=f"e{t}_{c}")
                nc.scalar.activation(
                    e[:, :, off:], sc[:, :, off:],
                    mybir.ActivationFunctionType.Exp, scale=SCALE,
                )
                if t == 0 and c < 2:
                    nc.vector.tensor_mul(e, e, maskF_sb[:, c])
                elif c - 4 * t >= 0:
                    nc.vector.tensor_mul(
                        e[:, :, off:off + P], e[:, :, off:off + P], maskT_sb
                    )
                acc2 = state[t]["acc"]
                if c < 2:
                    nc.vector.tensor_copy(acc2[c], e)
                else:
                    a = acc2[c % 2]
                    nc.vector.tensor_add(a[:, :, off:], a[:, :, off:], e[:, :, off:])
                state[t]["e"][c] = (e, off)

            def chunk_pv(t, c, nch):
                e, off = state[t]["e"].pop(c)
                pv_ps = state[t]["pv"]
                for h in range(HPC):
                    nc.tensor.matmul(
                        pv_ps[h][:, off:],
                        lhsT=V_sb[:, c, h * P:(h + 1) * P],
                        rhs=e[:, h, off:],
                        start=(c == 0), stop=(c == nch - 1),
                        skip_group_check=True,
                    )

            def fold(t):
                # partition sums -> broadcast denominator [128, h, q]
                acc2 = state[t]["acc"]
                den = p2sc.tile([P, HPC, QT], F32, tag="sc", name=f"den{t}")
                for h in range(HPC):
                    for a in range(2):
                        nc.tensor.matmul(
                            den[:, h, :],
                            lhsT=ones_sb,
                            rhs=acc2[a][:, h, :],
                            start=(a == 0), stop=(a == 1),
                        )
                state[t]["den"] = den

            def norm(t):
                # reciprocal + normalize into A_sb; frees pv PSUM
                q0 = t * QT
                den = state[t]["den"]
                pv_ps = state[t]["pv"]
                rec = p2rec.tile([P, HPC, QT], F32, tag="rec", name=f"rec{t}")
                for h in range(HPC):
                    nc.vector.reciprocal_approx_fast(rec[:, h, :], den[:, h, :])
                for h in range(HPC):
                    nc.vector.tensor_mul(
                        A_sb[:, h, q0:q0 + QT], pv_ps[h], rec[:, h, :]
                    )

            def oproj_group(t, g):
                sub, htile = divmod(g, H // QT)
                i = t * (QT // P) + sub
                po = p2po.tile([P, QT], F32, tag="po", name=f"po{t}_{g}")
                for oc in range(HPC):
                    nc.tensor.matmul(
                        po,
                        lhsT=A_sb[:, oc, i * P:(i + 1) * P],
                        rhs=woT_sb[:, oc, htile * QT:(htile + 1) * QT],
                        start=(oc == 0), stop=(oc == HPC - 1),
                    )
                stg = p2st.tile([P, QT], BF16, tag="stg", name=f"stg{t}_{g}")
                if ncopy[0] % 16 < 11:
                    nc.vector.tensor_copy(stg, po)
                else:
                    nc.scalar.activation(
                        stg, po, mybir.ActivationFunctionType.Copy
                    )
                ncopy[0] += 1
                nc.sync.dma_start(
                    out[i * P:(i + 1) * P, htile * QT:(htile + 1) * QT], stg
                )

            prev = None
            for t in range(NQT):
                nch = 4 * t + 4
                state[t] = {
                    "pv": [
                        p2pv.tile([P, QT], F32, tag=f"pv{h}", name=f"pv{h}_{t}")
                        for h in range(HPC)
                    ],
                    "acc": [
                        p2acc.tile([P, HPC, QT], FP16, tag=f"acc{a}",
                                   name=f"acc{a}_{t}")
                        for a in range(2)
                    ],
                    "e": {},
                }
                if prev is None:
                    for c in range(nch):
                        chunk_front(t, c)
                        chunk_pv(t, c, nch)
                else:
                    # software pipeline: previous tile's epilogue and o_proj
                    # interleave with this tile's chunk stream
                    chunk_front(t, 0)
                    fold(prev)
                    chunk_front(t, 1)
                    norm(prev)
                    chunk_pv(t, 0, nch)
                    chunk_pv(t, 1, nch)
                    done = 0
                    for c in range(2, nch):
                        chunk_front(t, c)
                        chunk_pv(t, c, nch)
                        quota = (NOG * (c - 1)) // (nch - 2)
                        while done < quota:
                            oproj_group(prev, done)
                            done += 1
                    while done < NOG:
                        oproj_group(prev, done)
                        done += 1
                    del state[prev]
                prev = t

            fold(prev)
            norm(prev)
            for g in range(NOG):
                oproj_group(prev, g)


def build_program():
    """Build + compile the Bass program (same program for all 8 cores)."""
    global _PROGRAM
    if _PROGRAM is not None:
        return _PROGRAM
    nc = bacc.Bacc(
        "TRN2", target_bir_lowering=False, debug=False, enable_asserts=False
    )
    with tile.TileContext(nc) as tc:
        _build_body(tc)
    nc.compile()
    _PROGRAM = nc
    return nc


def make_in_maps(hidden_states, w_qkv, w_o):
    import ml_dtypes

    x = np.asarray(hidden_states, dtype=np.float32).reshape(S, H)
    w = np.asarray(w_qkv, dtype=np.float32)
    wo = np.asarray(w_o, dtype=np.float32)

    xT = np.ascontiguousarray(x.T).astype(ml_dtypes.bfloat16)    # [2048, 4096]

    # rope tables, [128, 2, 4096]: rows 0:64 and 64:128 both hold the
    # [64, S] table so the doubled layout lines up with [real; imag] dims.
    e = np.arange(0, DH, 2, dtype=np.float32) / np.float32(DH)
    inv_freq = (1.0 / np.power(np.float32(10000.0), e)).astype(np.float32)
    t = np.arange(S, dtype=np.float32)
    freqs = np.outer(t, inv_freq).astype(np.float32)     # [S, 64]
    cosT = np.cos(freqs).T                               # [64, S]
    sinT = np.sin(freqs).T
    rope = np.empty((P, 2, S), dtype=np.float32)
    rope[0:64, 0] = cosT
    rope[64:128, 0] = cosT
    rope[0:64, 1] = sinT
    rope[64:128, 1] = sinT

    # signed half-swap permutation: (J.T @ z)[d] = -z[64+d], [64+d] = +z[d]
    swapj = np.zeros((P, P), dtype=ml_dtypes.bfloat16)
    for d in range(64):
        swapj[64 + d, d] = -1.0
        swapj[d, 64 + d] = 1.0

    ones16 = np.ones((P, P), dtype=np.float16)

    # triangular in-block mask, shared by every diagonal 128x128 block:
    # keep (ki, qi) iff qi >= ki; duplicated per head
    ki = np.arange(P)[:, None]
    qi = np.arange(P)[None, :]
    mT = (qi >= ki).astype(np.float16)                   # [128, 128]
    maskT = np.repeat(mT[:, None, :], HPC, axis=1)       # [128, 2, 128]

    # full-width masks for q-tile 0 chunks 0,1: keep iff qi >= ki + 128j
    qi5 = np.arange(QT)[None, :]
    maskF = np.empty((P, 2, HPC, QT), dtype=np.float16)
    for j in range(2):
        m = (qi5 >= ki + P * j).astype(np.float16)       # [128, 512]
        for h in range(HPC):
            maskF[:, j, h, :] = m

    in_maps = []
    for c in range(NCORES):
        r0 = c * OLOC
        w_loc = np.concatenate(
            [
                w[r0:r0 + OLOC],
                w[NH * DH + r0:NH * DH + r0 + OLOC],
                w[2 * NH * DH + r0:2 * NH * DH + r0 + OLOC],
            ],
            axis=0,
        )                                                # [768, 2048]
        wqkvT_c = np.ascontiguousarray(w_loc.T).astype(ml_dtypes.bfloat16)
        woT_c = np.ascontiguousarray(
            wo[:, r0:r0 + OLOC].T
        ).astype(ml_dtypes.bfloat16)                     # [256, 2048]
        in_maps.append(
            {
                "xT": xT,
                "wqkvT": wqkvT_c,
                "woT": woT_c,
                "rope": rope,
                "swapj": swapj,
                "ones16": ones16,
                "maskT": maskT,
                "maskF": maskF,
            }
        )
    return in_maps


def run_cores(in_maps, trace=False, **kwargs):
    nc = build_program()
    return run_bass_kernel_spmd(
        nc, in_maps, list(range(NCORES)), trace=trace, **kwargs
    )


def kernel(hidden_states, w_qkv, w_o):
    in_maps = make_in_maps(hidden_states, w_qkv, w_o)
    res = run_cores(in_maps)
    acc = res.results[0]["out"].astype(np.float32)
    for c in range(1, NCORES):
        acc = acc + res.results[c]["out"].astype(np.float32)
    return acc.reshape(1, S, H)


# revision 5
# speedup vs baseline: 1.0220x; 1.0220x over previous
"""Trainium2 Bass kernel for a 16-head causal self-attention block.

Reference computation (B=1, S=4096, H=2048, 16 heads x 128 dim, fp32):
    qkv = x @ w_qkv.T            # [S, 6144]
    q, k = rope(q), rope(k)      # half-split rope
    attn = causal_softmax(q k^T / sqrt(128)) @ v
    out  = attn @ w_o.T          # [S, 2048]

Sharding: tensor-parallel over heads.  Each of the 8 cores owns 2 heads:
it computes its slice of the QKV projection (768 rows), attention for its
2 heads, and a partial o_proj ([S, 2048], bf16); the host sums the 8
partials in f32.

Dataflow per core (matmul operands bf16/fp16, accumulation/softmax fp32):
  phase 1 (QKV+rope), per s-tile of 512: w/x chunk DMAs interleaved so the
    first sweep starts as soon as pair 0 lands; Q, K, V matmul sweeps with
    rope fused into the Q/K epilogues (rotate_half = signed-permutation
    matmul on the PE); V stored fp16.
  phase 2 (attention + o_proj software-pipelined), per q-tile of 512:
    causal k-chunks of 128 with variable-width matmuls on the diagonal
    (blocks strictly above the diagonal are never computed), exp -> fp16,
    a single shared [128,128] triangular mask for diagonal blocks, softmax
    denominators accumulated in fp16 on DVE (2x mode) into two alternating
    accumulators, folded across partitions by a ones-matmul per tile;
    the previous tile's fold/normalize and o_proj groups are interleaved
    into the next tile's chunk stream so the PE never idles; o_proj
    staging copies are split DVE/ACT to balance engine load.
"""

import numpy as np

import concourse.bass as bass
import concourse.mybir as mybir
import concourse.tile as tile
from concourse import bacc
from concourse.bass_utils import run_bass_kernel_spmd

F32 = mybir.dt.float32
BF16 = mybir.dt.bfloat16
FP16 = mybir.dt.float16

S = 4096
H = 2048
DH = 128
NH = 16
NCORES = 8
HPC = NH // NCORES          # 2 heads per core
OLOC = HPC * DH             # 256 local o-channels per q/k/v group
P = 128
ST1 = 512                   # phase-1 s-tile width
NHT = H // P                # 16 h-chunks
QT = 512                    # phase-2 q-tile width
NQT = S // QT               # 8 q-tiles
NKC = S // P                # 32 k-chunks
NOG = (QT // P) * (H // QT)  # 16 o_proj groups per q-tile
SCALE = 1.0 / float(np.sqrt(np.float32(DH)))

_PROGRAM = None


def _build_body(tc):
    nc = tc.nc

    xT = nc.dram_tensor("xT", [H, S], BF16, kind="ExternalInput").ap()
    wqkvT = nc.dram_tensor("wqkvT", [H, 3 * OLOC], BF16, kind="ExternalInput").ap()
    woT = nc.dram_tensor("woT", [OLOC, H], BF16, kind="ExternalInput").ap()
    rope = nc.dram_tensor("rope", [P, 2, S], F32, kind="ExternalInput").ap()
    swapj = nc.dram_tensor("swapj", [P, P], BF16, kind="ExternalInput").ap()
    ones16 = nc.dram_tensor("ones16", [P, P], FP16, kind="ExternalInput").ap()
    maskT = nc.dram_tensor("maskT", [P, HPC, P], FP16, kind="ExternalInput").ap()
    maskF = nc.dram_tensor("maskF", [P, 2, HPC, QT], FP16, kind="ExternalInput").ap()
    out = nc.dram_tensor("out", [S, H], BF16, kind="ExternalOutput").ap()

    xT_v = xT.rearrange("(t p) s -> p t s", p=P)        # [128, 16, 4096]
    wq_v = wqkvT.rearrange("(t p) o -> p t o", p=P)     # [128, 16, 768]
    woT_v = woT.rearrange("(t p) h -> p t h", p=P)      # [128, 2, 2048]

    with tc.tile_pool(name="resident", bufs=1) as resident:
        # d-major Q^T/K^T: [128 d, head, s]; s-major V: [128 s, k-chunk, 256]
        QT_sb = resident.tile([P, HPC, S], BF16)
        KT_sb = resident.tile([P, HPC, S], BF16)
        V_sb = resident.tile([P, NKC, OLOC], FP16)
        A_sb = resident.tile([P, HPC, S], BF16)          # normalized attn^T
        woT_sb = resident.tile([P, HPC, H], BF16)
        ones_sb = resident.tile([P, P], FP16)
        maskT_sb = resident.tile([P, HPC, P], FP16)
        maskF_sb = resident.tile([P, 2, HPC, QT], FP16)

        # ---------------- phase 1: QKV projection + rope ----------------
        with (
            tc.tile_pool(name="p1w", bufs=1) as p1w,
            tc.tile_pool(name="p1x", bufs=36) as p1x,
            tc.tile_pool(name="p1tab", bufs=3) as p1tab,
            tc.tile_pool(name="p1tmp", bufs=4) as p1tmp,
            tc.tile_pool(name="p1ps", bufs=1, space="PSUM") as p1ps,
            tc.tile_pool(name="p1rot", bufs=2, space="PSUM") as p1rot,
        ):
            J_sb = p1w.tile([P, P], BF16)
            nc.sync.dma_start(J_sb, swapj)
            wT_sb = p1w.tile([P, NHT, 3 * OLOC], BF16)

            def rope_block(blk, dst, cos, sin):
                t1 = p1tmp.tile([P, ST1], F32, tag="t1", name="t1")
                t2 = p1tmp.tile([P, ST1], BF16, tag="t2", name="t2")
                nc.vector.tensor_mul(t1, blk, cos)
                nc.vector.tensor_mul(t2, blk, sin)
                rot = p1rot.tile([P, ST1], F32, tag="rot", name="rot")
                nc.tensor.matmul(rot, lhsT=J_sb, rhs=t2, start=True, stop=True)
                nc.vector.tensor_add(dst, t1, rot)

            for st in range(S // ST1):
                s0 = st * ST1
                tab = p1tab.tile([P, 2, ST1], F32, tag="tab")
                nc.sync.dma_start(tab, rope[:, :, s0:s0 + ST1])
                xts = []
                for ht in range(NHT):
                    if st == 0:
                        nc.sync.dma_start(wT_sb[:, ht, :], wq_v[:, ht, :])
                    xt = p1x.tile([P, ST1], BF16, tag="xt", name=f"xt{st}_{ht}")
                    nc.sync.dma_start(xt, xT_v[:, ht, s0:s0 + ST1])
                    xts.append(xt)
                if st == 0:
                    # prefetch phase-2 constants while phase 1 computes
                    nc.sync.dma_start(ones_sb, ones16)
                    nc.sync.dma_start(maskT_sb, maskT)
                    nc.sync.dma_start(maskF_sb, maskF)
                    nc.sync.dma_start(woT_sb, woT_v)
                cos = tab[:, 0, :]
                sin = tab[:, 1, :]

                # --- Q sweep + rope (DMA-paced on s-tile 0) ---
                pq = p1ps.tile([P, 2 * ST1], F32, tag="pq", name=f"pq{st}")
                for ht in range(NHT):
                    for h in range(HPC):
                        nc.tensor.matmul(
                            pq[:, h * ST1:(h + 1) * ST1],
                            lhsT=wT_sb[:, ht, h * P:(h + 1) * P],
                            rhs=xts[ht],
                            start=ht == 0, stop=ht == NHT - 1,
                        )
                for h in range(HPC):
                    rope_block(pq[:, h * ST1:(h + 1) * ST1],
                               QT_sb[:, h, s0:s0 + ST1], cos, sin)

                # --- K sweep + rope ---
                pk = p1ps.tile([P, 2 * ST1], F32, tag="pk", name=f"pk{st}")
                for ht in range(NHT):
                    for h in range(HPC):
                        nc.tensor.matmul(
                            pk[:, h * ST1:(h + 1) * ST1],
                            lhsT=wT_sb[:, ht, OLOC + h * P:OLOC + (h + 1) * P],
                            rhs=xts[ht],
                            start=ht == 0, stop=ht == NHT - 1,
                        )
                for h in range(HPC):
                    rope_block(pk[:, h * ST1:(h + 1) * ST1],
                               KT_sb[:, h, s0:s0 + ST1], cos, sin)

                # --- V sweep ---
                # [128, 1024]: s-subs 0,1 share zero-region/bank 0; 2,3 share 1
                pv = p1ps.tile([P, 2 * ST1], F32, tag="pv", name=f"pv{st}")
                for ht in range(NHT):
                    for sub in range(ST1 // P):
                        nc.tensor.matmul(
                            pv[:, sub * OLOC:(sub + 1) * OLOC],
                            lhsT=xts[ht][:, sub * P:(sub + 1) * P],
                            rhs=wT_sb[:, ht, 2 * OLOC:3 * OLOC],
                            start=(ht == 0) and sub % 2 == 0,
                            stop=(ht == NHT - 1) and sub % 2 == 1,
                        )
                for sub in range(ST1 // P):
                    nc.scalar.activation(
                        V_sb[:, st * (ST1 // P) + sub, :],
                        pv[:, sub * OLOC:(sub + 1) * OLOC],
                        mybir.ActivationFunctionType.Copy,
                    )

        # ---------------- phase 2: attention + o_proj ----------------
        with (
            tc.tile_pool(name="p2e", bufs=4) as p2e,
            tc.tile_pool(name="p2acc", bufs=2) as p2acc,
            tc.tile_pool(name="p2rec", bufs=2) as p2rec,
            tc.tile_pool(name="p2st", bufs=4) as p2st,
            tc.tile_pool(name="p2sc", bufs=2, space="PSUM") as p2sc,
            tc.tile_pool(name="p2pv", bufs=1, space="PSUM") as p2pv,
            tc.tile_pool(name="p2po", bufs=2, space="PSUM") as p2po,
        ):
            state = {}
            ncopy = [0]

            def chunk_off(t, c):
                j = c - 4 * t
                if j < 0 or (t == 0 and c < 2):
                    return 0
                return P * j

            def chunk_front(t, c):
                # scores -> exp -> (mask) -> denominator accumulate
                q0 = t * QT
                off = chunk_off(t, c)
                sc = p2sc.tile([P, HPC, QT], F32, tag="sc", name=f"sc{t}_{c}")
                for h in range(HPC):
                    nc.tensor.matmul(
                        sc[:, h, off:],
                        lhsT=KT_sb[:, h, c * P:(c + 1) * P],
                        rhs=QT_sb[:, h, q0 + off:q0 + QT],
                        start=True, stop=True,
                    )
                e = p2e.tile([P, HPC, QT], FP16, tag="e", name=f"e{t}_{c}")
                nc.scalar.activation(
                    e[:, :, off:], sc[:, :, off:],
                    mybir.ActivationFunctionType.Exp, scale=SCALE,
                )
                if t == 0 and c < 2:
                    nc.vector.tensor_mul(e, e, maskF_sb[:, c])
                elif c - 4 * t >= 0:
                    nc.vector.tensor_mul(
                        e[:, :, off:off + P], e[:, :, off:off + P], maskT_sb
                    )
                acc2 = state[t]["acc"]
                if c < 2:
                    nc.vector.tensor_copy(acc2[c], e)
                else:
                    a = acc2[c % 2]
                    nc.vector.tensor_add(a[:, :, off:], a[:, :, off:], e[:, :, off:])
                state[t]["e"][c] = (e, off)

            def chunk_pv(t, c, nch):
                e, off = state[t]["e"].pop(c)
                pv_ps = state[t]["pv"]
                for h in range(HPC):
                    nc.tensor.matmul(
                        pv_ps[h][:, off:],
                        lhsT=V_sb[:, c, h * P:(h + 1) * P],
                        rhs=e[:, h, off:],
                        start=(c == 0), stop=(c == nch - 1),
                        skip_group_check=True,
                    )

            def fold(t):
                # partition sums -> broadcast denominator, one po-ring bank
                # per head (keeps the sc ring free for score double-buffering)
                acc2 = state[t]["acc"]
                den = [
                    p2po.tile([P, QT], F32, tag="po", name=f"den{t}_{h}")
                    for h in range(HPC)
                ]
                for h in range(HPC):
                    for a in range(2):
                        nc.tensor.matmul(
                            den[h],
                            lhsT=ones_sb,
                            rhs=acc2[a][:, h, :],
                            start=(a == 0), stop=(a == 1),
                        )
                state[t]["den"] = den

            def norm(t):
                # reciprocal + normalize into A_sb; frees pv PSUM
                q0 = t * QT
                den = state[t]["den"]
                pv_ps = state[t]["pv"]
                rec = p2rec.tile([P, HPC, QT], F32, tag="rec", name=f"rec{t}")
                for h in range(HPC):
                    nc.vector.reciprocal_approx_fast(rec[:, h, :], den[h])
                for h in range(HPC):
                    nc.vector.tensor_mul(
                        A_sb[:, h, q0:q0 + QT], pv_ps[h], rec[:, h, :]
                    )

            def oproj_group(t, g):
                sub, htile = divmod(g, H // QT)
                i = t * (QT // P) + sub
                po = p2po.tile([P, QT], F32, tag="po", name=f"po{t}_{g}")
                for oc in range(HPC):
                    nc.tensor.matmul(
                        po,
                        lhsT=A_sb[:, oc, i * P:(i + 1) * P],
                        rhs=woT_sb[:, oc, htile * QT:(htile + 1) * QT],
                        start=(oc == 0), stop=(oc == HPC - 1),
                    )
                stg = p2st.tile([P, QT], BF16, tag="stg", name=f"stg{t}_{g}")
                if ncopy[0] % 16 < 11:
                    nc.vector.tensor_copy(stg, po)
                else:
                    nc.scalar.activation(
                        stg, po, mybir.ActivationFunctionType.Copy
                    )
                ncopy[0] += 1
                nc.sync.dma_start(
                    out[i * P:(i + 1) * P, htile * QT:(htile + 1) * QT], stg
                )

            # global o_proj queue: (tile, group) pairs of completed tiles,
            # emitted at most one per chunk so the PE load per chunk stays
            # uniform (~1 chunk of score/PV + 1 o_proj group)
            pending = []
            prev = None
            for t in range(NQT):
                nch = 4 * t + 4
                state[t] = {
                    "pv": [
                        p2pv.tile([P, QT], F32, tag=f"pv{h}", name=f"pv{h}_{t}")
                        for h in range(HPC)
                    ],
                    "acc": [
                        p2acc.tile([P, HPC, QT], FP16, tag=f"acc{a}",
                                   name=f"acc{a}_{t}")
                        for a in range(2)
                    ],
                    "e": {},
                }
                if prev is None:
                    for c in range(nch):
                        chunk_front(t, c)
                        chunk_pv(t, c, nch)
                else:
                    # software pipeline: previous tile's epilogue and the
                    # o_proj queue interleave with this tile's chunk stream
                    chunk_front(t, 0)
                    fold(prev)
                    chunk_front(t, 1)
                    norm(prev)
                    chunk_pv(t, 0, nch)
                    chunk_pv(t, 1, nch)
                    pending.extend((prev, g) for g in range(NOG))
                    del state[prev]
                    for c in range(2, nch):
                        chunk_front(t, c)
                        chunk_pv(t, c, nch)
                        if pending:
                            oproj_group(*pending.pop(0))
                prev = t

            fold(prev)
            norm(prev)
            pending.extend((prev, g) for g in range(NOG))
            for tg in pending:
                oproj_group(*tg)


def build_program():
    """Build + compile the Bass program (same program for all 8 cores)."""
    global _PROGRAM
    if _PROGRAM is not None:
        return _PROGRAM
    nc = bacc.Bacc(
        "TRN2", target_bir_lowering=False, debug=False, enable_asserts=False
    )
    with tile.TileContext(nc) as tc:
        _build_body(tc)
    nc.compile()
    _PROGRAM = nc
    return nc


def make_in_maps(hidden_states, w_qkv, w_o):
    import ml_dtypes

    x = np.asarray(hidden_states, dtype=np.float32).reshape(S, H)
    w = np.asarray(w_qkv, dtype=np.float32)
    wo = np.asarray(w_o, dtype=np.float32)

    xT = np.ascontiguousarray(x.T).astype(ml_dtypes.bfloat16)    # [2048, 4096]

    # rope tables, [128, 2, 4096]: rows 0:64 and 64:128 both hold the
    # [64, S] table so the doubled layout lines up with [real; imag] dims.
    e = np.arange(0, DH, 2, dtype=np.float32) / np.float32(DH)
    inv_freq = (1.0 / np.power(np.float32(10000.0), e)).astype(np.float32)
    t = np.arange(S, dtype=np.float32)
    freqs = np.outer(t, inv_freq).astype(np.float32)     # [S, 64]
    cosT = np.cos(freqs).T                               # [64, S]
    sinT = np.sin(freqs).T
    rope = np.empty((P, 2, S), dtype=np.float32)
    rope[0:64, 0] = cosT
    rope[64:128, 0] = cosT
    rope[0:64, 1] = sinT
    rope[64:128, 1] = sinT

    # signed half-swap permutation: (J.T @ z)[d] = -z[64+d], [64+d] = +z[d]
    swapj = np.zeros((P, P), dtype=ml_dtypes.bfloat16)
    for d in range(64):
        swapj[64 + d, d] = -1.0
        swapj[d, 64 + d] = 1.0

    ones16 = np.ones((P, P), dtype=np.float16)

    # triangular in-block mask, shared by every diagonal 128x128 block:
    # keep (ki, qi) iff qi >= ki; duplicated per head
    ki = np.arange(P)[:, None]
    qi = np.arange(P)[None, :]
    mT = (qi >= ki).astype(np.float16)                   # [128, 128]
    maskT = np.repeat(mT[:, None, :], HPC, axis=1)       # [128, 2, 128]

    # full-width masks for q-tile 0 chunks 0,1: keep iff qi >= ki + 128j
    qi5 = np.arange(QT)[None, :]
    maskF = np.empty((P, 2, HPC, QT), dtype=np.float16)
    for j in range(2):
        m = (qi5 >= ki + P * j).astype(np.float16)       # [128, 512]
        for h in range(HPC):
            maskF[:, j, h, :] = m

    in_maps = []
    for c in range(NCORES):
        r0 = c * OLOC
        w_loc = np.concatenate(
            [
                w[r0:r0 + OLOC],
                w[NH * DH + r0:NH * DH + r0 + OLOC],
                w[2 * NH * DH + r0:2 * NH * DH + r0 + OLOC],
            ],
            axis=0,
        )                                                # [768, 2048]
        wqkvT_c = np.ascontiguousarray(w_loc.T).astype(ml_dtypes.bfloat16)
        woT_c = np.ascontiguousarray(
            wo[:, r0:r0 + OLOC].T
        ).astype(ml_dtypes.bfloat16)                     # [256, 2048]
        in_maps.append(
            {
                "xT": xT,
                "wqkvT": wqkvT_c,
                "woT": woT_c,
                "rope": rope,
                "swapj": swapj,
                "ones16": ones16,
                "maskT": maskT,
                "maskF": maskF,
            }
        )
    return in_maps


def run_cores(in_maps, trace=False, **kwargs):
    nc = build_program()
    return run_bass_kernel_spmd(
        nc, in_maps, list(range(NCORES)), trace=trace, **kwargs
    )


def kernel(hidden_states, w_qkv, w_o):
    in_maps = make_in_maps(hidden_states, w_qkv, w_o)
    res = run_cores(in_maps)
    acc = res.results[0]["out"].astype(np.float32)
    for c in range(1, NCORES):
        acc = acc + res.results[c]["out"].astype(np.float32)
    return acc.reshape(1, S, H)
